# revision 1
# baseline (speedup 1.0000x reference)
"""Causal multi-head attention on 8 trn2 NeuronCores.

Problem: B=4, S=2048, D=1024, H=16 heads (HD=64), causal softmax attention
with out-projection + bias.

Sharding (tensor-parallel over heads, data-parallel over batch):
  core c -> batch b = c // 2, head half = c % 2 (8 of 16 heads, 512 dims).
  Every core runs the IDENTICAL program on different data:
    - xt   : x[b].T                  [1024, 2048] (host pre-transposed)
    - wq/wk/wv : W[:, half slice]    [1024, 512]
    - wot  : Wo[:, half slice].T     [512, 1024]
    - bo   : bias on even cores, zeros on odd cores  [1, 1024]
  Core output: partial out-projection [2048, 1024]; host sums the two
  partials per batch at unshard time (row-parallel out_proj reduction).

Kernel (per core), flash-style with transposed scores:
  QT = wq.T @ x.T   [512, 2048]   (lhsT = wq natural layout)
  KT likewise; V = x @ wv [2048, 512] augmented with a ones column per head
  (the 65th row of the ctx matmul then yields the softmax denominator Z).
  scoresT[k, q] per head = KT_h^T-slice @ QT_h  -> psum [128 keys, q]
  causal mask added as data (affine_select-generated -1e9 tile), exp on ACT
  (scale = 1/8 folded in), ctx^T accumulated over key tiles, normalized by
  1/Z (DVE reciprocal + gpsimd partition_broadcast), then out-projection
  with bias applied via a rank-1 (ones x bo) matmul.
"""

import os

import numpy as np

import concourse.bass as bass
import concourse.mybir as mybir
import concourse.tile as tile
from concourse import bacc
from concourse.bass_utils import run_bass_kernel_spmd

B, S, D, H = 4, 2048, 1024, 16
HD = 64          # head dim
DL = 512         # local head dims per core (8 heads)
HH = 8           # local heads
P = 128
N_CORES = 8
QC = 512         # q chunk (moving free dim)
N_QC = S // QC   # 4
N_KT = S // P    # 16 key tiles
N_DI = D // P    # 8
N_DL = DL // P   # 4
N_ST = S // P    # 16 seq tiles
VW = HD + 1      # 65: V columns + ones column

F32 = mybir.dt.float32
# float32r streams fp32 through the PE at 1 cycle/row (vs 4 for float32)
# at slightly relaxed precision. Overridable for accuracy experiments.
_MM_DT = {
    "f32": mybir.dt.float32,
    "f32r": mybir.dt.float32r,
    "bf16": mybir.dt.bfloat16,
}[os.environ.get("MHA_MM_DT", "f32r")]


MMT = _MM_DT  # dtype for every tile that feeds the PE


_ldw_q = [0]


def _ldw(nc, dst, src_ap):
    """DMA-load a matmul-operand tile, casting f32 -> MMT when needed.

    Only gpsimd (SWDGE) DMAs may cast; walrus requires fp32r matmul inputs
    to be produced as rounded fp32r, so a plain bitcast is not enough.
    Round-robin over the 8 SWDGE queues so the casts run in parallel.
    """
    if MMT == F32:
        nc.sync.dma_start(dst, src_ap)
    else:
        nc.gpsimd.dma_start(dst, src_ap)


def build_nc():
    nc = bacc.Bacc("TRN2", target_bir_lowering=False, debug=False,
                   num_devices=N_CORES, num_swdge_queues=4)
    xt = nc.dram_tensor("xt", [D, S], F32, kind="ExternalInput").ap()
    wq = nc.dram_tensor("wq", [D, DL], F32, kind="ExternalInput").ap()
    wk = nc.dram_tensor("wk", [D, DL], F32, kind="ExternalInput").ap()
    wv = nc.dram_tensor("wv", [D, DL], F32, kind="ExternalInput").ap()
    wot = nc.dram_tensor("wot", [DL, O_], F32, kind="ExternalInput").ap()
    bo = nc.dram_tensor("bo", [1, O_], F32, kind="ExternalInput").ap()
    out = nc.dram_tensor("out", [S, O_], F32, kind="ExternalOutput").ap()

    repeat = int(os.environ.get("MHA_REPEAT", "1"))
    hwloop = int(os.environ.get("MHA_HWLOOP", "0"))
    with tile.TileContext(nc) as tc:
        if hwloop > 1:
            with tc.For_i(0, hwloop, 1):
                _emit(nc, tc, xt, wq, wk, wv, wot, bo, out)
        else:
            for _ in range(repeat):
                _emit(nc, tc, xt, wq, wk, wv, wot, bo, out)
    nc.compile()
    return nc


O_ = 1024  # output dims (full)


def _emit(nc, tc, xt, wq, wk, wv, wot, bo, out):
    from contextlib import ExitStack

    Exp = mybir.ActivationFunctionType.Exp
    add = mybir.AluOpType.add
    mult = mybir.AluOpType.mult

    with ExitStack() as ctx:
        # ---- small constants -------------------------------------------------
        consts = ctx.enter_context(tc.tile_pool(name="consts", bufs=1))
        # memset cannot write f32r; stage ones in f32 and cast-copy
        ones1 = consts.tile([1, P], MMT, tag="ones1")
        ones_f = consts.tile([P, P], F32, tag="ones_f")
        ones_v = ones_f[:, 0:HH]
        nc.gpsimd.memset(ones_f[:], 1.0)
        if MMT == F32:
            nc.gpsimd.memset(ones1[:], 1.0)
        else:
            nc.vector.tensor_copy(ones1[:], ones_f[0:1, :])
        bo_sb = consts.tile([1, O_], MMT, tag="bo_sb")
        _ldw(nc, bo_sb[:], bo[:])

        # ---- persistent activation storage ----------------------------------
        qt_pool = ctx.enter_context(tc.tile_pool(name="qt", bufs=1))
        kt_pool = ctx.enter_context(tc.tile_pool(name="kt", bufs=1))
        v_pool = ctx.enter_context(tc.tile_pool(name="v", bufs=1))
        qt_t = [qt_pool.tile([P, S], MMT, name=f"qt{j}", tag=f"qt{j}") for j in range(N_DL)]
        kt_t = [kt_pool.tile([P, S], MMT, name=f"kt{j}", tag=f"kt{j}") for j in range(N_DL)]
        v_t = [v_pool.tile([P, HH * VW], MMT, name=f"v{i}", tag=f"v{i}") for i in range(N_ST)]

        # ---- phase 1: projections -------------------------------------------
        with tc.tile_pool(name="xtp", bufs=1) as xt_pool, \
             tc.tile_pool(name="wp", bufs=1) as w_pool, \
             tc.tile_pool(name="pps", bufs=4, space="PSUM") as pps:
            xt_sb = []
            for i in range(N_DI):
                xti = xt_pool.tile([P, S], MMT, name=f"xt{i}", tag=f"xt{i}")
                _ldw(nc, xti[:], xt[i * P:(i + 1) * P, :])
                xt_sb.append(xti)

            for w_dram, dst, nm in ((wq, qt_t, "q"), (wk, kt_t, "k")):
                w_sb = []
                for i in range(N_DI):
                    wi = w_pool.tile([P, DL], MMT, name=f"w{nm}{i}", tag=f"w{i}")
                    _ldw(nc, wi[:], w_dram[i * P:(i + 1) * P, :])
                    w_sb.append(wi)
                for dq in range(N_DL):
                    for qc in range(N_QC):
                        ps = pps.tile([P, QC], F32, tag="pp")
                        for di in range(N_DI):
                            nc.tensor.matmul(
                                ps[:],
                                (w_sb[di][:, dq * P:(dq + 1) * P]),
                                (xt_sb[di][:, qc * QC:(qc + 1) * QC]),
                                start=(di == 0), stop=(di == N_DI - 1))
                        nc.vector.tensor_copy(dst[dq][:, qc * QC:(qc + 1) * QC], ps[:])

            wv_sb = []
            for i in range(N_DI):
                wvi = w_pool.tile([P, DL], MMT, name=f"wv{i}", tag=f"w{i}")
                _ldw(nc, wvi[:], wv[i * P:(i + 1) * P, :])
                wv_sb.append(wvi)
            for st in range(N_ST):
                ps = pps.tile([P, DL], F32, tag="pp")
                for di in range(N_DI):
                    nc.tensor.matmul(
                        ps[:],
                        (xt_sb[di][:, st * P:(st + 1) * P]),
                        (wv_sb[di][:]),
                        start=(di == 0), stop=(di == N_DI - 1))
                # scatter 8 heads x 64 into the 65-wide per-head slots
                vv = v_t[st].rearrange("p (h w) -> p h w", w=VW)
                nc.vector.tensor_copy(vv[:, :, 0:HD],
                                      ps.rearrange("p (h w) -> p h w", w=HD))
                nc.vector.tensor_copy(vv[:, :, HD:VW],
                                      ones_v.rearrange("p (h o) -> p h o", o=1))

        if os.environ.get("MHA_PHASES") == "1":
            return
        # ---- phase 2: attention ---------------------------------------------
        # ct is allocated only now so it reuses the space freed by xt/w
        ct_pool = ctx.enter_context(tc.tile_pool(name="ct", bufs=1))
        ct_t = [ct_pool.tile([P, S], MMT, name=f"ct{j}", tag=f"ct{j}") for j in range(N_DL)]
        # prefetch wot during attention so out-proj never waits on its DMA
        wot_pool = ctx.enter_context(tc.tile_pool(name="wotp", bufs=1))
        wot_sb = []
        for j in range(N_DL):
            wj = wot_pool.tile([P, O_], MMT, name=f"wot{j}", tag=f"wot{j}")
            _ldw(nc, wj[:], wot[j * P:(j + 1) * P, :])
            wot_sb.append(wj)
        with tc.tile_pool(name="mskp", bufs=1) as mskp, \
             tc.tile_pool(name="exp", bufs=int(os.environ.get("MHA_EXBUFS", "8"))) as exp_pool, \
             tc.tile_pool(name="zp", bufs=2) as z_pool, \
             tc.tile_pool(name="scps", bufs=3, space="PSUM") as sc_pool, \
             tc.tile_pool(name="ctxps", bufs=2, space="PSUM") as ctx_pool:
            # mask[k, j*512 + q] = 0 if q >= k + 128*j else -1e9
            # (j = key-tile position within the 4-tile diagonal band)
            mask = mskp.tile([P, 4 * QC], F32, tag="mask")
            nc.gpsimd.memset(mask[:], 0.0)
            m3 = mask.rearrange("p (j q) -> p j q", q=QC)
            nc.gpsimd.affine_select(
                out=m3, in_=m3,
                pattern=[[-P, 4], [1, QC]],
                compare_op=mybir.AluOpType.is_ge,
                fill=-1e9, base=0, channel_multiplier=-1)

            # Flattened, software-pipelined emission. The PE executes in
            # program order, so per-group "scores -> exp -> ctx" emission
            # stalls the PE on ACT every group. Instead ctx matmuls trail
            # their score-group by STAGGER groups: the exp runs while the
            # PE streams later score-groups.
            #
            # Diagonal-band trim: for a key tile kt whose band offset
            # d = kt*128 - qc*512 is > 0, only q in [d, 512) is unmasked;
            # the scores / exp / mask / ctx for q < d are skipped entirely.
            STAGGER = int(os.environ.get("MHA_STAGGER", "6"))
            units = []
            for qc in range(N_QC):
                for h in range(HH):
                    ng = 2 * (qc + 1)
                    for g in range(ng):
                        units.append((qc, h, g, ng))

            state = {}   # (qc, h) -> dict(sc/ex per live group, ctx_ps)

            NOMASK = bool(os.environ.get("MHA_NOMASK"))
            NOEXP = bool(os.environ.get("MHA_NOEXP"))
            NOCTX = bool(os.environ.get("MHA_NOCTX"))

            def emit_scores(u):
                qc, h, g, ng = u
                hr = slice(HD * (h % 2), HD * (h % 2) + HD)
                ht = h // 2
                sc = sc_pool.tile([P, 2 * QC], F32, tag="sc")
                ex = exp_pool.tile([P, 2 * QC], MMT, tag="ex")
                offs = []
                for j in (0, 1):
                    kt = 2 * g + j
                    d = max(0, kt * P - qc * QC)   # masked q prefix width
                    offs.append(d)
                    nc.tensor.matmul(
                        sc[:, j * QC + d:(j + 1) * QC],
                        kt_t[ht][hr, kt * P:(kt + 1) * P],
                        qt_t[ht][hr, qc * QC + d:(qc + 1) * QC],
                        start=True, stop=True)
                    if d and not NOMASK:  # partial causal tile: mask the sub-diagonal part
                        nc.vector.tensor_tensor(
                            sc[:, j * QC + d:(j + 1) * QC],
                            sc[:, j * QC + d:(j + 1) * QC],
                            mask[:, j * QC + d:(j + 1) * QC] if g == 2 * qc
                            else mask[:, (j + 2) * QC + d:(j + 3) * QC], add)
                    elif g == 2 * qc and j == 0 and not NOMASK:
                        # kt == 4*qc: d == 0 but still the diagonal tile
                        nc.vector.tensor_tensor(
                            sc[:, 0:QC], sc[:, 0:QC], mask[:, 0:QC], add)
                if NOEXP:
                    pass
                elif offs[0] == offs[1] == 0:
                    nc.scalar.activation(ex[:], sc[:], Exp, scale=0.125)
                else:
                    for j in (0, 1):
                        d = offs[j]
                        nc.scalar.activation(
                            ex[:, j * QC + d:(j + 1) * QC],
                            sc[:, j * QC + d:(j + 1) * QC], Exp, scale=0.125)
                state[(qc, h, g)] = (sc, ex, offs)

            def emit_ctx(u):
                if NOCTX:
                    state.pop(u[:3], None)
                    return
                qc, h, g, ng = u
                ht = h // 2
                if g == 0:
                    state[(qc, h, "ctx")] = ctx_pool.tile([P, QC], F32, tag="ctx", name=f"ctx{qc}_{h}")
                ctx_ps = state[(qc, h, "ctx")]
                sc, ex, offs = state.pop((qc, h, g))
                nkt = 2 * ng
                for j in (0, 1):
                    kt = 2 * g + j
                    d = offs[j]
                    nc.tensor.matmul(
                        ctx_ps[0:VW, d:QC],
                        v_t[kt][:, h * VW:(h + 1) * VW],
                        ex[:, j * QC + d:(j + 1) * QC],
                        start=(kt == 0), stop=(kt == nkt - 1))
                if g == ng - 1:
                    ctx_ps = state.pop((qc, h, "ctx"))
                    hr = slice(HD * (h % 2), HD * (h % 2) + HD)
                    rec = z_pool.tile([1, QC], F32, tag="rec")
                    nc.vector.reciprocal(rec[:], ctx_ps[HD:VW, :])
                    rzb = z_pool.tile([HD, QC], F32, tag="rzb")
                    nc.gpsimd.partition_broadcast(rzb[:], rec[:])
                    nc.vector.tensor_tensor(
                        ct_t[ht][hr, qc * QC:(qc + 1) * QC],
                        ctx_ps[0:HD, :], rzb[:], mult)

            for i, u in enumerate(units):
                emit_scores(u)
                if i >= STAGGER:
                    emit_ctx(units[i - STAGGER])
            for u in units[-STAGGER:]:
                emit_ctx(u)

        if os.environ.get("MHA_PHASES") == "2":
            return
        # ---- phase 3: out-projection ----------------------------------------
        with tc.tile_pool(name="outp", bufs=3) as out_pool, \
             tc.tile_pool(name="ops", bufs=4, space="PSUM") as ops:
            for qt in range(N_ST):
                ob = out_pool.tile([P, O_], F32, tag="ob")
                for oc in range(2):
                    ps = ops.tile([P, QC], F32, tag="op")
                    for dl in range(N_DL):
                        nc.tensor.matmul(
                            ps[:],
                            (ct_t[dl][:, qt * P:(qt + 1) * P]),
                            (wot_sb[dl][:, oc * QC:(oc + 1) * QC]),
                            start=(dl == 0), stop=False)
                    nc.tensor.matmul(
                        ps[:], (ones1[:]),
                        (bo_sb[:, oc * QC:(oc + 1) * QC]),
                        start=False, stop=True)
                    nc.vector.tensor_copy(ob[:, oc * QC:(oc + 1) * QC], ps[:])
                nc.sync.dma_start(out[qt * P:(qt + 1) * P, :], ob[:])


_NC_CACHE = None


def _get_nc():
    global _NC_CACHE
    if _NC_CACHE is None:
        _NC_CACHE = build_nc()
    return _NC_CACHE


def make_in_maps(x, Wq, Wk, Wv, Wo, bo):
    in_maps = []
    xts = [np.ascontiguousarray(x[b].T) for b in range(B)]
    zeros_bo = np.zeros((1, O_), np.float32)
    for c in range(N_CORES):
        b, half = c // 2, c % 2
        d0 = half * DL
        in_maps.append({
            "xt": xts[b],
            "wq": np.ascontiguousarray(Wq[:, d0:d0 + DL]),
            "wk": np.ascontiguousarray(Wk[:, d0:d0 + DL]),
            "wv": np.ascontiguousarray(Wv[:, d0:d0 + DL]),
            "wot": np.ascontiguousarray(Wo[:, d0:d0 + DL].T),
            "bo": bo.reshape(1, O_).astype(np.float32) if half == 0 else zeros_bo,
        })
    return in_maps


def kernel(x, Wq, Wk, Wv, Wo, bo):
    x = np.asarray(x, np.float32)
    Wq = np.asarray(Wq, np.float32)
    Wk = np.asarray(Wk, np.float32)
    Wv = np.asarray(Wv, np.float32)
    Wo = np.asarray(Wo, np.float32)
    bo = np.asarray(bo, np.float32)
    nc = _get_nc()
    in_maps = make_in_maps(x, Wq, Wk, Wv, Wo, bo)
    res = run_bass_kernel_spmd(nc, in_maps, core_ids=list(range(N_CORES)))
    out = np.empty((B, S, O_), np.float32)
    for b in range(B):
        out[b] = res.results[2 * b]["out"] + res.results[2 * b + 1]["out"]
    return out

